# revision 31
# baseline (speedup 1.0000x reference)
"""Trainium2 Bass/Tile kernel for GroupNorm + MultiHeadAttention + proj + residual.

Reference computation (per batch b):
    xf  = x[b] reshaped (C, T=H*W)
    xn  = GroupNorm32(xf) * norm_w + norm_b          (per-channel affine)
    qkv = qkv_w @ xn + qkv_b                         (3C, T)
    per head h (8 heads, hd=64):
        scores = (q*s)^T (k*s), s = hd**-0.25        (T, T)
        P = softmax(scores, axis=-1)
        h_out = P @ v^T  -> (hd, T)
    y   = proj_w @ h + proj_b + xf                   (C, T)

Distribution: pure data parallel over batch: 16 batches / 8 cores = 2 per core.

Speed strategy (vs the f32r baseline):
  - scores, PV and proj matmuls run as fp8 DoubleRow (0.5 cyc/row, 2 k-tiles
    per instruction).  QKV stays f32r for accuracy.
  - Q/K are emitted via a host-side weight-row permutation directly into the
    DoubleRow layout [128, 2, T] (4 heads x 32 partitions; dim1 = c-half).
  - softmax: exp(s - 3) with the constant shift folded into the activation
    bias (max score 8.0 -> max exp ~148 fits fp8e4's 240).  The exp work is
    split across three engines:
      'A' chunks: ACT true Exp -> fp8e4 P
      'D'/'P' chunks: DVE/Pool Schraudolph (y = s*a + b -> int8, bitcast
        fp8e5): one tensor_scalar op, ~3% P error, cancels in softmax.
  - V^T carries a ones column so PV row 64 accumulates the softmax
    denominator for free; 1/sum via reciprocal_approx_fast; broadcast to 64
    channel rows with a K=1 f32r matmul; H evac fuses normalize+fp8-quant.
  - v-bias is folded into proj bias on the host (h_norm = pv*r + vb  =>
    y += proj_w @ vb, a constant).
  - PSUM: "sT" tag [128,1024]x3 bufs (6 banks) shared by scores psums and all
    small matmul psums (qkv/proj/rbc/gn); "pv" [65,1024]x1 (2 banks).
"""

import numpy as np

import concourse.bass as bass
import concourse.mybir as mybir
import concourse.tile as tile
from concourse import bacc

F32 = mybir.dt.float32
F32R = mybir.dt.float32r
BF16 = mybir.dt.bfloat16
FP8 = mybir.dt.float8e4      # ml_dtypes.float8_e4m3 (IEEE, max 240)
FP8E5 = mybir.dt.float8e5
I8 = mybir.dt.int8
AF = mybir.ActivationFunctionType
OP = mybir.AluOpType
DR = mybir.MatmulPerfMode.DoubleRow

B, C, HH, WW = 16, 512, 32, 32
T = HH * WW            # 1024
NH, HD = 8, 64         # heads, head dim
N_CORES = 8
BPC = B // N_CORES     # batches per core = 2
CT = C // 128          # 4 channel tiles
ST = T // 128          # 8 s-chunks / t-tiles
GROUPS = 32
GS = C // GROUPS       # 16 channels per group
GPT = 128 // GS        # 8 groups per 128-channel tile
EPS = 1e-5
SCALE = float(HD) ** -0.25

# softmax shift: exp(s - M_SHIFT); scores (fp8 q/k) span [-8.75, 8.01]
M_SHIFT = 3.0
# Schraudolph fp8e5 constants: y = (s - M)*A5 + B5 -> int8 -> bitcast e5m2
SCHR_A = 4.0 / float(np.log(2.0))            # 5.7708
SCHR_B = 72.0                                # y in [4.2, 100.9] for s above
SCHR_BIAS = SCHR_B - M_SHIFT * SCHR_A        # folded constant

# per-head exp engine: 'A' ACT exp->e4, 'D'/'P' DVE/Pool schraudolph->e5.
# e5 must be uniform per head (the e5 constant carries a 2^c scale that only
# cancels within one softmax row); bf16 schraudolph ('Db'/'Pb') is scale-free
# and may therefore replace individual 'A' pairs for engine balance (PV pays
# the non-DoubleRow bf16 rate for those pairs).
HEAD_ENG = ['A', 'A', 'A', 'A', 'A', 'D', 'D', 'P']
PAIR_OVERRIDE = {}
# bf16 schraudolph constants (c=0: matches exp(s - M_SHIFT) scale exactly)
SCHR16_A = 128.0 / float(np.log(2.0))
SCHR16_B = 127.0 * 128 - 0.043 * 128 + 0.5
PW_SPLIT = False  # proj_w as hi+lo fp8 pair (2x DR matmuls, host-side split)


def _build_body(ctx, tc, d):
    nc = tc.nc
    assert BPC == 2

    const = ctx.enter_context(tc.tile_pool(name="const", bufs=1))
    sb = ctx.enter_context(tc.tile_pool(name="sb", bufs=1))
    ps = ctx.enter_context(tc.tile_pool(name="ps", space="PSUM", bufs=1))

    # ---- tiny consts first (they gate groupnorm), then x, all on SP -----
    # (gpsimd-issued DMAs cost ~1us of Pool-engine SWDGE time each; SP is
    # otherwise idle)
    gmask = const.tile([128, GPT], F32, name="gmask")
    nc.sync.dma_start(out=gmask, in_=d["gmask"])
    bmask = const.tile([GPT, 128], F32, name="bmask")
    nc.sync.dma_start(out=bmask, in_=d["bmask"])
    nwc = const.tile([128, CT], F32, name="nwc")
    nc.sync.dma_start(out=nwc, in_=d["nw_cols"])
    nbc = const.tile([128, CT], F32, name="nbc")
    nc.sync.dma_start(out=nbc, in_=d["nb_cols"])
    qkb = const.tile([128, 8], F32, name="qkb")
    nc.sync.dma_start(out=qkb, in_=d["qk_bias_cols"])

    S = [dict() for _ in range(BPC)]
    for b in range(BPC):
        S[b]["x"] = []
        for k in range(CT):
            xk = sb.tile([128, T], F32, name=f"x{b}_{k}", tag=f"x{k}", bufs=2)
            for half in range(2):
                nc.sync.dma_start(
                    out=xk[:, half * 512:(half + 1) * 512],
                    in_=d["x"][b, k * 128:(k + 1) * 128, half * 512:(half + 1) * 512],
                )
            S[b]["x"].append(xk)
    zeros = const.tile([128, 1], F32, name="zeros")
    nc.vector.memset(zeros, 0.0)
    mshift = const.tile([128, 1], F32, name="mshift")
    nc.vector.memset(mshift, -M_SHIFT)
    ones64 = const.tile([1, 64], F32, name="ones64")
    nc.vector.memset(ones64, 1.0)

    # qkv weights: fp8 DoubleRow layout [kk][p, i, o] (contraction pairs)
    qkv_wT = []
    for kk in range(2):
        w1 = const.tile([128, 2, 3 * C], FP8, name=f"qkv_wT{kk}")
        nc.sync.dma_start(out=w1, in_=d["qkv_wT"][kk])
        qkv_wT.append(w1)

    # proj weights fp8 (hi, optional lo) laid out [128, 2, C]
    pw8 = []
    for kk in range(2):
        w2 = const.tile([128, 2, C], FP8, name=f"pw8_{kk}")
        nc.sync.dma_start(out=w2, in_=d["pw8"][kk])
        pw8.append(w2)
    pw8_lo = []
    if PW_SPLIT:
        for kk in range(2):
            w3 = const.tile([128, 2, C], FP8, name=f"pw8lo_{kk}")
            nc.sync.dma_start(out=w3, in_=d["pw8_lo"][kk])
            pw8_lo.append(w3)
    pbc = const.tile([128, CT], F32, name="pbc")
    nc.sync.dma_start(out=pbc, in_=d["pb_cols"])

    # ---- emitters -------------------------------------------------------

    def emit_gn_stats(b):
        """GroupNorm stats -> per-channel A/B tiles (tiny).  Runs at startup
        for BOTH batches so the ACT Sqrt (different table set from Exp)
        never forces a mid-run table switch."""
        x = S[b]["x"]
        ge = sb.tile([GPT, CT, 2], F32, name=f"ge{b}", tag="ge", bufs=2)
        # batch the op classes so the DVE chain doesn't serialize per-k
        # against the PE group matmuls (startup critical path)
        sts, s2s = [], []
        for k in range(CT):
            st = sb.tile([128, 2, 6], F32, name=f"st{b}_{k}", tag=f"st{k}",
                         bufs=2)
            nc.vector.bn_stats(out=st[:, 0, :], in_=x[k][:, 0:512])
            nc.vector.bn_stats(out=st[:, 1, :], in_=x[k][:, 512:1024])
            sts.append(st)
        for k in range(CT):
            mv = sb.tile([128, 2], F32, name=f"mv{b}_{k}", tag=f"mv{k}", bufs=2)
            nc.vector.bn_aggr(out=mv, in_=sts[k])
            s2 = sb.tile([128, 2], F32, name=f"s2{b}_{k}", tag=f"s2{k}", bufs=2)
            nc.vector.tensor_copy(out=s2[:, 0:1], in_=mv[:, 0:1])
            nc.vector.scalar_tensor_tensor(
                out=s2[:, 1:2], in0=mv[:, 0:1], scalar=mv[:, 0:1],
                in1=mv[:, 1:2], op0=OP.mult, op1=OP.add,
            )
            s2s.append(s2)
        gps = []
        for k in range(CT):
            gp = ps.tile([GPT, 2], F32, name=f"gp{b}_{k}", tag="sT", bufs=3)
            nc.tensor.matmul(gp, gmask, s2s[k], start=True, stop=True)
            gps.append(gp)
        for k in range(CT):
            nc.vector.tensor_copy(out=ge[:, k, :], in_=gps[k])

        gstats = sb.tile([GPT, CT, 2], F32, name=f"gstats{b}", tag="gstats", bufs=2)
        gvar = sb.tile([GPT, CT], F32, name=f"gvar{b}", tag="gvar", bufs=2)
        nc.vector.tensor_mul(gvar, ge[:, :, 0], ge[:, :, 0])
        nc.vector.tensor_sub(gvar, ge[:, :, 1], gvar)
        nc.vector.tensor_scalar_add(gvar, gvar, EPS)
        nc.scalar.activation(out=gvar, in_=gvar, func=AF.Sqrt, bias=zeros[0:GPT, :])
        nc.vector.reciprocal(out=gstats[:, :, 1], in_=gvar)
        nc.vector.tensor_copy(out=gstats[:, :, 0], in_=ge[:, :, 0])

        S[b]["AB"] = []
        for k in range(CT):
            cps = ps.tile([128, 2], F32, name=f"cps{b}_{k}", tag="sT", bufs=3)
            nc.tensor.matmul(cps, bmask, gstats[:, k, :], start=True, stop=True)
            A = sb.tile([128, 1], F32, name=f"A{b}_{k}", tag=f"A{k}", bufs=2)
            Bc = sb.tile([128, 1], F32, name=f"B{b}_{k}", tag=f"B{k}", bufs=2)
            nc.vector.tensor_mul(A, cps[:, 1:2], nwc[:, k:k + 1])
            nc.vector.tensor_mul(Bc, cps[:, 0:1], A)
            nc.vector.tensor_sub(Bc, nbc[:, k:k + 1], Bc)
            S[b]["AB"].append((A, Bc))

    def emit_gn_apply(b, k, xn_eng='A'):
        """xn[k] = x[k]*A + B -> fp8, into the DR pair tile [128, 2, T]."""
        A, Bc = S[b]["AB"][k]
        kk, i = k // 2, k % 2
        key = f"xnp{kk}"
        if key not in S[b]:
            S[b][key] = sb.tile([128, 2, T], FP8, name=f"xnp{b}_{kk}",
                                tag=key, bufs=2)
        xnk = S[b][key][:, i, :]
        if xn_eng == 'A':
            nc.scalar.activation(
                out=xnk, in_=S[b]["x"][k], func=AF.Identity, bias=Bc, scale=A)
        elif xn_eng == 'D':
            nc.vector.tensor_scalar(
                out=xnk, in0=S[b]["x"][k], scalar1=A, scalar2=Bc,
                op0=OP.mult, op1=OP.add)
        else:
            nc.gpsimd.tensor_scalar(
                out=xnk, in0=S[b]["x"][k], scalar1=A, scalar2=Bc,
                op0=OP.mult, op1=OP.add)

    def emit_qk_block(b, blk):
        """One QK psum block -> fp8 interleaved Q/K tile half.

        blk 0..3 = Q (g, i); blk 4..7 = K (g, i); g = (blk%4)//2, i = blk%2.
        """
        isK = blk >= 4
        g, i = (blk % 4) // 2, blk % 2
        key = ("ka" if isK else "qa") + str(g)
        if key not in S[b]:
            S[b][key] = sb.tile([128, 2, T], FP8, name=f"{key}_{b}",
                                tag=key, bufs=2)
        dst = S[b][key]
        mm = ps.tile([128, T], F32, name=f"qk_ps{b}_{blk}", tag="sT", bufs=3)
        for n in range(2):
            for kk in range(2):
                nc.tensor.matmul(
                    mm[:, n * 512:(n + 1) * 512],
                    qkv_wT[kk][:, :, blk * 128:(blk + 1) * 128],
                    S[b][f"xnp{kk}"][:, :, n * 512:(n + 1) * 512],
                    start=(kk == 0), stop=(kk == 1),
                    perf_mode=DR,
                )
        nc.gpsimd.tensor_scalar(
            out=dst[:, i, :], in0=mm,
            scalar1=qkb[:, blk:blk + 1], scalar2=None, op0=OP.add)

    def emit_v_block(b, pr):
        """V^T for t-chunk pair pr -> fp8 vt pair tile (with ones cols)."""
        key = f"vt{pr}"
        if key not in S[b]:
            S[b][key] = sb.tile([128, 2, NH, HD + 1], FP8, name=f"{key}_{b}",
                                tag=key, bufs=2)
        vt = S[b][key]
        mm = ps.tile([128, 2, 512], F32, name=f"v_ps{b}_{pr}", tag="sT", bufs=3)
        for j in range(2):
            mt = 2 * pr + j
            for kk in range(2):
                nc.tensor.matmul(
                    mm[:, j, :],
                    S[b][f"xnp{kk}"][:, :, mt * 128:(mt + 1) * 128],
                    qkv_wT[kk][:, :, 2 * C:3 * C],
                    start=(kk == 0), stop=(kk == 1),
                    perf_mode=DR,
                )
        nc.gpsimd.tensor_copy(
            out=vt[:, :, :, 0:HD],
            in_=mm.rearrange("p j (h d) -> p j h d", h=NH))
        nc.gpsimd.memset(vt[:, :, :, HD:HD + 1], 1.0)

    def emit_scores(b, h, sc):
        """DoubleRow scores^T for (head, s-chunk) -> sT psum [128, T]."""
        g, a = h // 4, h % 4
        qa = S[b][f"qa{g}"]
        ka = S[b][f"ka{g}"]
        sT_ps = ps.tile([128, T], F32, name=f"sT{b}_{h}_{sc}", tag="sT", bufs=3)
        for n in range(2):
            nc.tensor.matmul(
                sT_ps[:, n * 512:(n + 1) * 512],
                ka[32 * a:32 * a + 32, :, sc * 128:(sc + 1) * 128],
                qa[32 * a:32 * a + 32, :, n * 512:(n + 1) * 512],
                start=True, stop=True, perf_mode=DR,
                tile_position=(32 * a, 0),
            )
        S[b][f"sT{h}_{sc}"] = sT_ps

    def pair_mode(h, j):
        return PAIR_OVERRIDE.get((h, j), HEAD_ENG[h])

    def emit_exp(b, h, sc):
        """exp chunk -> half of the pT pair tile, engine per pair mode."""
        j = sc // 2
        eng = pair_mode(h, j)
        key = f"pT{h}_{j}"
        if key not in S[b]:
            if eng == 'A':
                S[b][key] = sb.tile([128, 2, T], FP8, name=f"pTa{b}_{h}_{j}",
                                    tag="pTa", bufs=8)
            elif eng in ('D', 'P'):
                S[b][key] = sb.tile([128, 2, T], I8, name=f"pT{eng}{b}_{h}_{j}",
                                    tag=f"pT{eng}", bufs=9 if eng == "D" else 5)
            else:
                S[b][key] = sb.tile([128, 2, T], mybir.dt.int16,
                                    name=f"pTb{b}_{h}_{j}", tag="pTb", bufs=4)
        pT = S[b][key]
        sT_ps = S[b].pop(f"sT{h}_{sc}")
        if eng == 'A':
            nc.scalar.activation(
                out=pT[:, sc % 2, :], in_=sT_ps, func=AF.Exp,
                bias=mshift, scale=1.0)
        elif eng in ('D', 'P'):
            e = nc.vector if eng == 'D' else nc.gpsimd
            e.tensor_scalar(
                out=pT[:, sc % 2, :], in0=sT_ps,
                scalar1=float(SCHR_A), scalar2=float(SCHR_BIAS),
                op0=OP.mult, op1=OP.add)
        else:
            e = nc.vector if eng == 'Db' else nc.gpsimd
            e.tensor_scalar(
                out=pT[:, sc % 2, :], in0=sT_ps,
                scalar1=float(SCHR16_A),
                scalar2=float(SCHR16_B - M_SHIFT * SCHR16_A),
                op0=OP.mult, op1=OP.add)

    def emit_pv(b, h, j):
        """DoubleRow PV accumulate for sc-pair j of head h."""
        if j == 0:
            S[b][f"pv{h}"] = ps.tile([HD + 1, T], F32, name=f"pv{b}_{h}",
                                     tag="pv", bufs=1)
        pv_ps = S[b][f"pv{h}"]
        vt = S[b][f"vt{j}"]
        pT = S[b].pop(f"pT{h}_{j}")
        eng = pair_mode(h, j)
        if eng == 'A':
            rhs, pm = pT, DR
        elif eng in ('D', 'P'):
            rhs, pm = pT.bitcast(FP8E5), DR
        else:
            rhs, pm = pT.bitcast(BF16), None   # bf16: 2 plain matmuls/k-tile
        for n in range(2):
            if pm is DR:
                nc.tensor.matmul(
                    pv_ps[:, n * 512:(n + 1) * 512],
                    vt[:, :, h, :],
                    rhs[:, :, n * 512:(n + 1) * 512],
                    start=(j == 0), stop=(j == 3),
                    perf_mode=DR, skip_group_check=True,
                )
            else:
                for jj in range(2):
                    nc.tensor.matmul(
                        pv_ps[:, n * 512:(n + 1) * 512],
                        vt[:, jj, h, :],
                        rhs[:, jj, n * 512:(n + 1) * 512],
                        start=(j == 0 and n == 0 and jj == 0 and False),
                        stop=(j == 3 and n == 1 and jj == 1),
                        skip_group_check=True,
                    )

    def emit_hu_evac(b, h):
        """pv -> SBUF f32 [65, T] (rows 0-63 = unnormalized H, row 64 =
        softmax denominator).  One op frees the pv psum slot ~1.5us after
        the last PV, killing the serial per-head drain chain."""
        pv_ps = S[b].pop(f"pv{h}")
        hu = sb.tile([HD + 1, T], F32, name=f"hu{b}_{h}", tag="hu", bufs=3)
        nc.gpsimd.tensor_copy(out=hu, in_=pv_ps)
        S[b][f"hu{h}"] = hu

    def emit_denom_half(b, h, n):
        """1/rowsum half from the SBUF h_un row."""
        hu = S[b][f"hu{h}"]
        if n == 0:
            S[b][f"rbf{h}"] = sb.tile([1, T], F32, name=f"r{b}_{h}",
                                      tag="rbf", bufs=2)
        rbf = S[b][f"rbf{h}"]
        sl = slice(n * 512, (n + 1) * 512)
        nc.vector.reciprocal_approx_fast(out=rbf[:, sl], in_=hu[HD:HD + 1, sl])

    def emit_rbc_half(b, h, n):
        """broadcast 1/sum over 64 channel rows (K=1 f32r matmul)."""
        rbf = S[b][f"rbf{h}"]
        if n == 0:
            S[b][f"rbc{h}"] = ps.tile([64, T], F32, name=f"rbc{b}_{h}",
                                      tag="sT", bufs=3)
        rbc = S[b][f"rbc{h}"]
        nc.tensor.matmul(
            rbc[:, n * 512:(n + 1) * 512], ones64.bitcast(F32R),
            rbf.bitcast(F32R)[:, n * 512:(n + 1) * 512],
            start=True, stop=True,
        )

    def emit_hnorm(b, h):
        """normalize + fp8 H evac (one [64, T] op)."""
        hu = S[b][f"hu{h}"]
        kk, i, lo = h // 4, (h % 4) // 2, 64 * (h % 2)
        key = f"ht{kk}"
        if key not in S[b]:
            S[b][key] = sb.tile([128, 2, T], FP8, name=f"{key}_{b}",
                                tag=key, bufs=2)
        ht = S[b][key]
        nc.gpsimd.tensor_tensor(
            out=ht[lo:lo + 64, i, :],
            in0=hu[0:64, :],
            in1=S[b][f"rbc{h}"], op=OP.mult,
        )
        del S[b][f"rbc{h}"]
        del S[b][f"hu{h}"]

    def emit_proj(b, m, phase=None):
        """proj output tile m.  phase 0: kk=0 partial matmuls (needs only
        ht0); phase 1 (or None=both): kk=1 + evac + store."""
        key = f"y{m}"
        if phase != 1:
            pj = ps.tile([128, T], F32, name=f"pj{b}_{m}", tag="sT", bufs=3)
            S[b][f"pj{m}"] = pj
            ht0 = S[b]["ht0"]
            for n in range(2):
                nc.tensor.matmul(
                    pj[:, n * 512:(n + 1) * 512],
                    pw8[0][:, :, m * 128:(m + 1) * 128],
                    ht0[:, :, n * 512:(n + 1) * 512],
                    start=True, stop=False,
                    perf_mode=DR, skip_group_check=True,
                )
            if phase == 0:
                return
        pj = S[b][f"pj{m}"]
        ht1 = S[b]["ht1"]
        for n in range(2):
            nc.tensor.matmul(
                pj[:, n * 512:(n + 1) * 512],
                pw8[1][:, :, m * 128:(m + 1) * 128],
                ht1[:, :, n * 512:(n + 1) * 512],
                start=False, stop=True,
                perf_mode=DR, skip_group_check=True,
            )
        if key not in S[b]:
            S[b][key] = sb.tile([128, T], F32, name=f"y{b}_{m}",
                                tag=key, bufs=2)
        y = S[b][key]
        nc.gpsimd.scalar_tensor_tensor(
            out=y, in0=pj,
            scalar=pbc[:, m:m + 1],
            in1=S[b]["x"][m],
            op0=OP.add, op1=OP.add,
        )
        nc.sync.dma_start(
            out=d["out"][b, m * 128:(m + 1) * 128, :],
            in_=y,
        )

    # ---- schedule: weave driver ----------------------------------------
    # One chunk per step.  A-head chunks alternate 1:1 with front-loaded
    # D/P-head chunks so all three exp engines work concurrently; a strict
    # FIFO tail state machine pumps pv -> stg/recip -> rbc -> hnorm pieces
    # one per step (half-T granularity pipelines the drain chain); QKV / V /
    # proj blocks ride a background queue.
    A_HEADS, D_HEADS, P_HEADS = (0, 1, 2, 3, 4), (5, 6), (7,)
    TAIL_ORDER = [0, 1, 5, 2, 3, 7, 4, 6]
    PATTERN = "AADAAPAD"   # 5A + 2D + 1P per rep: aligns A-chunks to
                           # A-freed psum slots (bufs=3) most of the time

    def weave_chunks(b):
        A = [(b, h, sc) for h in A_HEADS for sc in range(ST)]
        D = [(b, h, sc) for h in D_HEADS for sc in range(ST)]
        P = [(b, h, sc) for h in P_HEADS for sc in range(ST)]
        q = {'A': A, 'D': D, 'P': P}
        out = []
        while any(q.values()):
            for c in PATTERN:
                if q[c]:
                    out.append(q[c].pop(0))
        return out

    # tail piece list: (min steps after previous piece, emit fn)
    def tail_pieces(b, h):
        return [
            (0, lambda: ([emit_pv(b, h, j) for j in range(4)],
                         emit_hu_evac(b, h))),
            (1, lambda: (emit_denom_half(b, h, 0),
                         emit_denom_half(b, h, 1))),
            (1, lambda: (emit_rbc_half(b, h, 0), emit_rbc_half(b, h, 1))),
            (1, lambda: emit_hnorm(b, h)),
        ]

    emit_gn_stats(0)
    for k, e in enumerate(('A', 'D', 'P', 'A')):
        emit_gn_apply(0, k, e)
    for blk in (0, 1, 4, 5):
        emit_qk_block(0, blk)
    emit_gn_stats(1)        # its ACT Sqrt lands before the first exp
    for blk in (2, 3, 6, 7):
        emit_qk_block(0, blk)

    bg = []
    bg += [(None, lambda pr=pr: emit_v_block(0, pr)) for pr in range(4)]
    bg += [(None, lambda k=k: emit_gn_apply(1, k, 'P'))
           for k in range(CT)]
    bg += [(None, lambda blk=blk: emit_qk_block(1, blk)) for blk in range(8)]
    N_BG0 = len(bg)
    bg += [(None, lambda pr=pr: emit_v_block(1, pr)) for pr in range(4)]
    # proj(0) gated on batch-0 tails complete (tail index 8)
    bg += [(8, lambda m=m: emit_proj(0, m)) for m in range(CT)]

    chunks = weave_chunks(0) + weave_chunks(1)
    tails = [(0, h) for h in TAIL_ORDER] + [(1, h) for h in TAIL_ORDER]
    emitted = {}
    ti = 0            # active tail index
    pieces = tail_pieces(*tails[0])
    pi = 0            # piece index within active tail
    cooldown = 0
    tails_done = 0
    bg_i = 0

    def pump_tail():
        nonlocal ti, pi, cooldown, pieces, tails_done
        if ti >= len(tails):
            return
        tb, th = tails[ti]
        if pi == 0 and emitted.get((tb, th), 0) < ST:
            return
        if cooldown > 0:
            cooldown -= 1
            return
        pieces[pi][1]()
        pi += 1
        if pi == len(pieces):
            ti += 1
            tails_done += 1
            pi = 0
            if ti < len(tails):
                pieces = tail_pieces(*tails[ti])
            cooldown = 0
        else:
            cooldown = max(0, pieces[pi][0] - 1)

    def pump_bg(budget=1):
        nonlocal bg_i
        for _ in range(budget):
            if bg_i >= len(bg):
                return
            gate, fn = bg[bg_i]
            if gate is not None and tails_done < gate:
                return
            fn()
            bg_i += 1

    for step, (b, h, sc) in enumerate(chunks):
        emit_scores(b, h, sc)
        pump_tail()
        emit_exp(b, h, sc)
        emitted[(b, h)] = emitted.get((b, h), 0) + 1
        in_b = step if step < 64 else step - 64
        if in_b < 16:
            pump_bg(1)              # front-load V blocks
        elif step % 2 == 0:
            pump_bg(1)
        if step == 63:
            while bg_i < N_BG0:     # batch-1 chunks need QK(1) complete
                pump_bg(1)

    # drain remaining tails and background (proj tiles), then proj(1)
    guard = 0
    while ti < len(tails) and guard < 200:
        pump_tail()
        pump_bg(1)
        guard += 1
    while bg_i < len(bg):
        pump_bg(1)
    for m in range(CT):
        emit_proj(1, m)


def build_nc():
    nc = bacc.Bacc("TRN2")
    d = {}
    d["x"] = nc.dram_tensor("x", [BPC, C, T], F32, kind="ExternalInput")[:]
    d["qkv_wT"] = nc.dram_tensor("qkv_wT", [2, 128, 2, 3 * C], FP8, kind="ExternalInput")[:]
    d["pw8"] = nc.dram_tensor("pw8", [2, 128, 2, C], FP8, kind="ExternalInput")[:]
    if PW_SPLIT:
        d["pw8_lo"] = nc.dram_tensor(
            "pw8_lo", [2, 128, 2, C], FP8, kind="ExternalInput")[:]
    d["qk_bias_cols"] = nc.dram_tensor(
        "qk_bias_cols", [128, 8], F32, kind="ExternalInput")[:]
    d["nw_cols"] = nc.dram_tensor("nw_cols", [128, CT], F32, kind="ExternalInput")[:]
    d["nb_cols"] = nc.dram_tensor("nb_cols", [128, CT], F32, kind="ExternalInput")[:]
    d["pb_cols"] = nc.dram_tensor("pb_cols", [128, CT], F32, kind="ExternalInput")[:]
    d["gmask"] = nc.dram_tensor("gmask", [128, GPT], F32, kind="ExternalInput")[:]
    d["bmask"] = nc.dram_tensor("bmask", [GPT, 128], F32, kind="ExternalInput")[:]
    d["out"] = nc.dram_tensor("out", [BPC, C, T], F32, kind="ExternalOutput")[:]

    from contextlib import ExitStack

    with tile.TileContext(nc) as tc:
        with ExitStack() as ctx:
            _build_body(ctx, tc, d)
    nc.finalize()
    return nc


def host_inputs(x, norm_w, norm_b, qkv_w, qkv_b, proj_w, proj_b):
    """Host-side constant preprocessing (numpy, cheap)."""
    import ml_dtypes
    f = np.float32
    E4 = ml_dtypes.float8_e4m3

    qkv_w = np.asarray(qkv_w, f).copy()
    qkv_b = np.asarray(qkv_b, f).copy()
    proj_w = np.asarray(proj_w, f)
    proj_b = np.asarray(proj_b, f)

    # reference row layout: head h rows [192h,192h+64) = q, +64 k, +128 v
    # fold q/k scale into weights+biases
    for h in range(NH):
        qkv_w[192 * h:192 * h + 128] *= f(SCALE)
        qkv_b[192 * h:192 * h + 128] *= f(SCALE)

    # QK psum block rows: blk<4 Q (g,i), blk>=4 K: row 32a+p ->
    #   qkv row 192*(4g+a) + (64 if K) + 32i + p
    perm = np.empty(3 * C, np.int64)
    for blk in range(8):
        isK = blk >= 4
        g, i = (blk % 4) // 2, blk % 2
        for a in range(4):
            hh = 4 * g + a
            base = 192 * hh + (64 if isK else 0) + 32 * i
            perm[blk * 128 + 32 * a: blk * 128 + 32 * a + 32] = \
                np.arange(base, base + 32)
    # V columns head-major: col 64h+dd -> row 192h+128+dd
    for h in range(NH):
        perm[1024 + 64 * h: 1024 + 64 * h + 64] = \
            np.arange(192 * h + 128, 192 * h + 192)
    wp = qkv_w[perm]
    bp = qkv_b[perm]

    # proj fp8 [kk][p, i, o] = proj_w[o, 256kk+128i+p]
    pwT = np.ascontiguousarray(proj_w.T)  # [c, o]
    pw8 = np.empty((2, 128, 2, C), ml_dtypes.float8_e4m3)
    pw8_lo = np.empty((2, 128, 2, C), ml_dtypes.float8_e4m3)
    for kk in range(2):
        for i in range(2):
            blk_ = pwT[256 * kk + 128 * i: 256 * kk + 128 * i + 128]
            hi = blk_.astype(E4)
            pw8[kk, :, i, :] = hi
            pw8_lo[kk, :, i, :] = (blk_ - hi.astype(f)).astype(E4)

    # v-bias fold: h_norm = pv*r + vb  =>  y += proj_w @ vb (constant).
    # H channel order is head-major (c = 64h+dd) = original channel order,
    # and bp[1024+c] is exactly the v bias of channel c.
    pb_eff = proj_b + proj_w @ bp[1024:1536]

    # fp8 DR layout: [kk][p, i, o] = w[o, 256kk+128i+p]
    wT = wp.T  # [c, o]
    w8 = np.empty((2, 128, 2, 3 * C), ml_dtypes.float8_e4m3)
    for kk in range(2):
        for i in range(2):
            w8[kk, :, i, :] = wT[256 * kk + 128 * i:
                                 256 * kk + 128 * i + 128].astype(E4)

    consts = {
        "qkv_wT": w8,
        "pw8": pw8,
        "qk_bias_cols": np.ascontiguousarray(bp[:1024].reshape(8, 128).T),
        "nw_cols": np.ascontiguousarray(np.asarray(norm_w, f).reshape(CT, 128).T),
        "nb_cols": np.ascontiguousarray(np.asarray(norm_b, f).reshape(CT, 128).T),
        "pb_cols": np.ascontiguousarray(pb_eff.astype(f).reshape(CT, 128).T),
    }
    if PW_SPLIT:
        consts["pw8_lo"] = pw8_lo
    gmask = np.zeros((128, GPT), f)
    for p in range(128):
        gmask[p, p // GS] = 1.0 / GS
    consts["gmask"] = gmask
    consts["bmask"] = np.ascontiguousarray((gmask.T > 0).astype(f))

    xs = np.ascontiguousarray(np.asarray(x, f).reshape(N_CORES, BPC, C, T))
    return xs, consts


_NC_CACHE = None


def kernel(x, norm_w, norm_b, qkv_w, qkv_b, proj_w, proj_b, num_heads=8, **_):
    from concourse.bass_utils import run_bass_kernel_spmd

    assert int(num_heads) == NH
    global _NC_CACHE
    if _NC_CACHE is None:
        _NC_CACHE = build_nc()
    nc = _NC_CACHE

    xs, consts = host_inputs(x, norm_w, norm_b, qkv_w, qkv_b, proj_w, proj_b)
    in_maps = [{"x": xs[i], **consts} for i in range(N_CORES)]
    res = run_bass_kernel_spmd(nc, in_maps, core_ids=list(range(N_CORES)))
    out = np.stack([res.results[i]["out"] for i in range(N_CORES)])
    return out.reshape(B, C, HH, WW)
